# revision 1
# baseline (speedup 1.0000x reference)
"""Grouped-Query Attention (B=2, S=2048, E=2048, H=16, KVH=4, D=128, causal)
as a Bass/Tile kernel on 8 Trainium2 NeuronCores.

Sharding: core c handles batch b=c//4 and kv-head-group g=c%4 (4 q heads +
1 kv head per core).  Out-proj is row-sharded: each core computes a partial
[E,S] (transposed) output; host sums the 4 partials per batch.

All activations/weights are kept TRANSPOSED (feature-major) so every matmul
has its contraction dim on SBUF partitions with no on-chip transposes
(except V, which needs [S,D] layout for the PV matmul - done via 16 cheap
PE transposes).  Scores are computed in [k,q] orientation so exp output
feeds the PV matmul directly; the softmax denominator comes from a
ones-vector matmul; normalization is applied after PV (deferred division).

Matmul inputs are bf16 (fp32 PSUM accumulation); measured end-to-end
absmax-relative error vs the fp32 reference is ~3.4e-3.
"""
import sys

for _p in ("/opt/trn_rl_repo", "/root/.axon_site/_ro/trn_rl_repo"):
    if _p not in sys.path:
        sys.path.append(_p)

import numpy as np
import ml_dtypes

import concourse.bass as bass
import concourse.mybir as mybir
import concourse.tile as tile
from concourse import bacc, bass_utils

B, S, E = 2, 2048, 2048
H, KVH = 16, 4
D = E // H              # 128
G = H // KVH            # 4 q heads per kv head
HPC = H // 8 * 2        # 4 q heads per core
SCALE = 1.0 / float(np.sqrt(D))
P = 128                 # partitions
NQ = 512                # q-group width (moving N)
BF = mybir.dt.bfloat16
F32 = mybir.dt.float32

_CACHE = {}


def _build():
    nc = bacc.Bacc("TRN2", target_bir_lowering=False, debug=False, num_devices=8)
    xT = nc.dram_tensor("xT", [E, S], BF, kind="ExternalInput").ap()
    wqkvT = nc.dram_tensor("wqkvT", [E, 768], BF, kind="ExternalInput").ap()
    woT = nc.dram_tensor("woT", [512, E], BF, kind="ExternalInput").ap()
    ident = nc.dram_tensor("ident", [P, P], BF, kind="ExternalInput").ap()
    ones = nc.dram_tensor("ones", [P, 1], BF, kind="ExternalInput").ap()
    tri = nc.dram_tensor("tri", [P, P], BF, kind="ExternalInput").ap()
    outT = nc.dram_tensor("outT", [E, S], F32, kind="ExternalOutput").ap()

    EK = E // P          # 16 contraction chunks for projections
    with tile.TileContext(nc) as tc:
        with tc.tile_pool(name="persist", bufs=1) as pp, \
             tc.tile_pool(name="probs", bufs=8) as prb, \
             tc.tile_pool(name="bcast", bufs=2) as bcp, \
             tc.tile_pool(name="small", bufs=2) as smp, \
             tc.tile_pool(name="outp", bufs=6) as outp, \
             tc.tile_pool(name="ps_proj", bufs=2, space="PSUM") as ps_proj, \
             tc.tile_pool(name="ps_s", bufs=3, space="PSUM") as ps_sp, \
             tc.tile_pool(name="ps_o", bufs=2, space="PSUM") as ps_op, \
             tc.tile_pool(name="ps_sum", bufs=1, space="PSUM") as ps_sump:

            # ---- load inputs ----
            xT_sb, wqkv_sb = [], []
            for i in range(EK):
                w = pp.tile([P, 768], BF, tag=f"wq{i}", name=f"wq{i}")
                nc.sync.dma_start(out=w, in_=wqkvT[i * P:(i + 1) * P, :])
                wqkv_sb.append(w)
                t = pp.tile([P, S], BF, tag=f"xT{i}", name=f"xT{i}")
                nc.sync.dma_start(out=t, in_=xT[i * P:(i + 1) * P, :])
                xT_sb.append(t)
            wo_sb = []
            for j in range(4):
                t = pp.tile([P, E], BF, tag=f"wo{j}")
                nc.sync.dma_start(out=t, in_=woT[j * P:(j + 1) * P, :])
                wo_sb.append(t)
            id_sb = pp.tile([P, P], BF, tag="ident")
            nc.sync.dma_start(out=id_sb, in_=ident)
            ones_sb = pp.tile([P, 1], BF, tag="ones")
            nc.sync.dma_start(out=ones_sb, in_=ones)
            tri_sb = pp.tile([P, P], BF, tag="tri")
            nc.sync.dma_start(out=tri_sb, in_=tri)

            # ---- phase 1: qkvT[768, S] = WqkvT.T @ xT ----
            qkv_sb = [pp.tile([P, S], BF, tag=f"qkv{m}", name=f"qkv{m}") for m in range(6)]
            _pools = [(ps_proj, "proj"), (ps_sp, "s"), (ps_op, "o")]
            for m in range(6):
                for ng in range(S // NQ):
                    _pl, _tg = _pools[(m * 4 + ng) % 3]
                    ps = _pl.tile([P, NQ], F32, tag=_tg)
                    for ke in range(EK):
                        nc.tensor.matmul(
                            ps,
                            wqkv_sb[ke][:, m * P:(m + 1) * P],
                            xT_sb[ke][:, ng * NQ:(ng + 1) * NQ],
                            start=(ke == 0), stop=(ke == EK - 1))
                    # alternate copy engine to split the work
                    nc.scalar.copy(qkv_sb[m][:, ng * NQ:(ng + 1) * NQ], ps)
            kT = qkv_sb[4]          # [D, S]
            vT = qkv_sb[5]          # [D, S]

            # ---- phase 1b: v natural layout [S, D] via PE transposes ----
            v_sb = []
            for kc in range(S // P):
                pst = ps_sp.tile([P, P], BF, tag="s")
                nc.tensor.transpose(pst, vT[:, kc * P:(kc + 1) * P], id_sb)
                vt = pp.tile([P, D], BF, tag=f"v{kc}")
                nc.vector.tensor_copy(vt, pst)
                v_sb.append(vt)

            # ---- phase 2: attention (4 heads, q-groups of 512, causal) ----
            attn_sb = [pp.tile([P, S], BF, tag=f"at{h}", name=f"at{h}") for h in range(HPC)]
            for g4 in range(S // NQ):
                for h in range(HPC):
                    qT_h = qkv_sb[h]
                    kmax = 4 * g4 + 4
                    ps_o = ps_op.tile([P, NQ], F32, tag="o")
                    ps_sum = ps_sump.tile([1, NQ], F32, tag="sum")
                    for kc in range(kmax):
                        ps_s = ps_sp.tile([P, NQ], F32, tag="s")
                        nc.tensor.matmul(
                            ps_s, kT[:, kc * P:(kc + 1) * P],
                            qT_h[:, g4 * NQ:(g4 + 1) * NQ],
                            start=True, stop=True)
                        pr = prb.tile([P, NQ], BF, tag="pr")
                        rel = P * (kc - 4 * g4)
                        if rel <= 0:
                            nc.scalar.activation(
                                pr, ps_s, mybir.ActivationFunctionType.Exp,
                                scale=SCALE)
                        else:
                            nc.gpsimd.memset(pr[:, :rel], 0.0)
                            nc.scalar.activation(
                                pr[:, rel:], ps_s[:, rel:],
                                mybir.ActivationFunctionType.Exp, scale=SCALE)
                        if rel >= 0:
                            # diagonal block: keep kp <= qp
                            nc.vector.tensor_mul(
                                pr[:, rel:rel + P], pr[:, rel:rel + P], tri_sb)
                        nc.tensor.matmul(ps_sum, ones_sb, pr,
                                         start=(kc == 0), stop=(kc == kmax - 1))
                        nc.tensor.matmul(ps_o, v_sb[kc], pr,
                                         start=(kc == 0), stop=(kc == kmax - 1))
                    rec = smp.tile([1, NQ], F32, tag="rec")
                    nc.vector.reciprocal(rec, ps_sum)
                    bc = bcp.tile([P, NQ], F32, tag="bc")
                    nc.gpsimd.partition_broadcast(bc, rec)
                    nc.vector.tensor_mul(
                        attn_sb[h][:, g4 * NQ:(g4 + 1) * NQ], ps_o, bc)

            # ---- phase 3: partial out-proj, outT[E, S] = WoT.T @ attnT ----
            for ng in range(S // NQ):
                for me in range(E // P):
                    ps = ps_proj.tile([P, NQ], F32, tag="proj")
                    for j in range(4):
                        nc.tensor.matmul(
                            ps, wo_sb[j][:, me * P:(me + 1) * P],
                            attn_sb[j][:, ng * NQ:(ng + 1) * NQ],
                            start=(j == 0), stop=(j == 3))
                    ot = outp.tile([P, NQ], F32, tag="out")
                    nc.vector.tensor_copy(ot, ps)
                    nc.sync.dma_start(
                        out=outT[me * P:(me + 1) * P, ng * NQ:(ng + 1) * NQ],
                        in_=ot)
    nc.compile()
    return nc


def _get_nc():
    if "nc" not in _CACHE:
        _CACHE["nc"] = _build()
    return _CACHE["nc"]


def kernel(x, Wq, Wk, Wv, Wo, _trace=False, _tmpdir=None):
    x = np.asarray(x, np.float32)
    Wq, Wk, Wv, Wo = (np.asarray(a, np.float32) for a in (Wq, Wk, Wv, Wo))
    nc = _get_nc()
    ident = np.eye(P, dtype=ml_dtypes.bfloat16)
    ones = np.ones((P, 1), dtype=ml_dtypes.bfloat16)
    tri = np.triu(np.ones((P, P), np.float32)).astype(ml_dtypes.bfloat16)
    from concurrent.futures import ThreadPoolExecutor
    with ThreadPoolExecutor(8) as tp:
        xT_bf = list(tp.map(
            lambda b: np.ascontiguousarray(x[b].T).astype(ml_dtypes.bfloat16),
            range(B)))

        def _core(c):
            b, g = c // 4, c % 4
            wqkv = np.concatenate(
                [Wq[512 * g:512 * (g + 1)],
                 Wk[128 * g:128 * (g + 1)],
                 Wv[128 * g:128 * (g + 1)]], axis=0)
            return {
                "xT": xT_bf[b],
                "wqkvT": np.ascontiguousarray(wqkv.T).astype(ml_dtypes.bfloat16),
                "woT": np.ascontiguousarray(
                    Wo[:, 512 * g:512 * (g + 1)].T).astype(ml_dtypes.bfloat16),
                "ident": ident, "ones": ones, "tri": tri,
            }
        in_maps = list(tp.map(_core, range(8)))
    res = bass_utils.run_bass_kernel_spmd(
        nc, in_maps, core_ids=list(range(8)), trace=_trace, tmpdir=_tmpdir)
    out = np.zeros((B, S, E), np.float32)
    for c in range(8):
        out[c // 4] += res.results[c]["outT"].T
    if _trace:
        return out, res
    return out



# revision 12
# speedup vs baseline: 1.2098x; 1.2098x over previous
"""Grouped-Query Attention (B=2, S=2048, E=2048, H=16, KVH=4, D=128, causal)
as a Bass/Tile kernel on 8 Trainium2 NeuronCores.

Sharding: core c handles batch b=c//4 and kv-head-group g=c%4 (4 q heads +
1 kv head per core).  Out-proj is row-sharded: each core computes a partial
[E,S] (transposed) output in fp16; host sums the 4 partials per batch.

All activations/weights are kept TRANSPOSED (feature-major, fp16) so every
matmul has its contraction dim on SBUF partitions.  V is re-laid out to
[S,D] via 16 PE transposes.  Scores are computed in [k,q] orientation so
the exp output feeds the PV matmul directly.

Performance structure (the PE executes its stream in order, so emission
order is the schedule):
 - fp16 everywhere (same PE speed as bf16, 4x the mantissa precision).
 - softmax denominators accumulated with vector/gpsimd adds and one
   gpsimd partition_all_reduce per (q-block, head) - no ones-matmuls.
 - causal diagonal blocks stream only the live columns.
 - projection contraction runs ke-outermost over 8 concurrent PSUM banks
   so the first pass streams at DMA arrival rate.
 - attention is exp(ACT)-bound locally, so out-proj / q-proj matmuls are
   interleaved into the attention stream at kc granularity to keep the
   PE fed while the scalar engine catches up.
 - output partials are written as fp16 (host accumulates in fp32).
"""
import sys

for _p in ("/opt/trn_rl_repo", "/root/.axon_site/_ro/trn_rl_repo"):
    if _p not in sys.path:
        sys.path.append(_p)

import numpy as np
import ml_dtypes

import concourse.bass as bass
import concourse.mybir as mybir
import concourse.tile as tile
from concourse import bacc, bass_utils
from concourse import bass_isa

B, S, E = 2, 2048, 2048
H, KVH = 16, 4
D = E // H              # 128
HPC = 4                 # q heads per core
SCALE = 1.0 / float(np.sqrt(D))
P = 128                 # partitions
NQ = 512                # q-group width
EK = E // P             # 16 contraction chunks for projections
F16 = mybir.dt.float16
F32 = mybir.dt.float32

_CACHE = {}


def _build():
    nc = bacc.Bacc("TRN2", target_bir_lowering=False, debug=False, num_devices=8)
    xT = nc.dram_tensor("xT", [E, S], F16, kind="ExternalInput").ap()
    wqT = nc.dram_tensor("wqT", [E, 512], F16, kind="ExternalInput").ap()
    wkvT = nc.dram_tensor("wkvT", [E, 256], F16, kind="ExternalInput").ap()
    woT = nc.dram_tensor("woT", [512, E], F16, kind="ExternalInput").ap()
    ident = nc.dram_tensor("ident", [P, P], F16, kind="ExternalInput").ap()
    tri = nc.dram_tensor("tri", [P, P], F16, kind="ExternalInput").ap()
    outT = nc.dram_tensor("outT", [E, S], F16, kind="ExternalOutput").ap()

    with tile.TileContext(nc) as tc:
        with tc.tile_pool(name="persist", bufs=1) as pp, \
             tc.tile_pool(name="probs", bufs=6) as prb, \
             tc.tile_pool(name="accp", bufs=3) as accp, \
             tc.tile_pool(name="sump", bufs=2) as sump, \
             tc.tile_pool(name="recp", bufs=2) as recp, \
             tc.tile_pool(name="outp", bufs=6) as outp, \
             tc.tile_pool(name="ps", bufs=8, space="PSUM") as psp:

            # ---- input DMAs (issue order == model stream order) ----
            id_sb = pp.tile([P, P], F16, tag="ident")
            nc.sync.dma_start(out=id_sb, in_=ident)
            wkv_sb = [pp.tile([P, 256], F16, tag=f"wkv{ke}", name=f"wkv{ke}")
                      for ke in range(EK)]
            xT_sb = [pp.tile([P, S], F16, tag=f"xT{ke}", name=f"xT{ke}")
                     for ke in range(EK)]
            nc.sync.dma_start(out=wkv_sb[0], in_=wkvT[0:P, :])
            # first x chunk in quarters so the first matmul starts early
            for q4 in range(4):
                nc.sync.dma_start(
                    out=xT_sb[0][:, q4 * NQ:(q4 + 1) * NQ],
                    in_=xT[0:P, q4 * NQ:(q4 + 1) * NQ])
            for ke in range(1, EK):
                nc.sync.dma_start(out=wkv_sb[ke],
                                  in_=wkvT[ke * P:(ke + 1) * P, :])
                nc.sync.dma_start(out=xT_sb[ke],
                                  in_=xT[ke * P:(ke + 1) * P, :])
            tri_sb = pp.tile([P, P], F16, tag="tri")
            nc.sync.dma_start(out=tri_sb, in_=tri)
            wq_sb = []
            for ke in range(EK):
                w = pp.tile([P, 512], F16, tag=f"wq{ke}", name=f"wq{ke}")
                nc.sync.dma_start(out=w, in_=wqT[ke * P:(ke + 1) * P, :])
                wq_sb.append(w)
            wo_sb = []
            for j in range(4):
                t = pp.tile([P, E], F16, tag=f"wo{j}")
                nc.sync.dma_start(out=t, in_=woT[j * P:(j + 1) * P, :])
                wo_sb.append(t)

            warm_ps = psp.tile([P, NQ], F32, tag="ps", name="warm")
            for _ in range(26):
                nc.tensor.matmul(warm_ps[:, 0:P], id_sb, id_sb,
                                 start=True, stop=True)

            kT = pp.tile([P, S], F16, tag="kT", name="kT")
            vT = pp.tile([P, S], F16, tag="vT", name="vT")
            qT_sb = [pp.tile([P, S], F16, tag=f"qT{h}", name=f"qT{h}")
                     for h in range(HPC)]
            attn_sb = [pp.tile([P, S], F16, tag=f"at{h}", name=f"at{h}")
                       for h in range(HPC)]
            v_sb = [pp.tile([P, D], F16, tag=f"v{kc}", name=f"v{kc}")
                    for kc in range(S // P)]

            # alternate proj epilogue copies between ACT and DVE
            _cp = [0]

            def copy_alt(dst, src):
                if _cp[0] % 2 == 0:
                    nc.scalar.copy(dst, src)
                else:
                    nc.vector.tensor_copy(dst, src)
                _cp[0] += 1

            def proj_pass(srcs, dsts):
                """ke-outer projection pass over len(srcs)*4 PSUM tiles.
                srcs: list of (weight_tiles, col_off); dsts: matching output
                tiles [P, S] written per ng block."""
                tiles = [(i, ng) for i in range(len(srcs)) for ng in range(4)]
                pss = [psp.tile([P, NQ], F32, tag="ps", name="pspj")
                       for _ in tiles]
                for ke in range(EK):
                    for t, (i, ng) in enumerate(tiles):
                        w, off = srcs[i]
                        nc.tensor.matmul(
                            pss[t],
                            w[ke][:, off:off + P],
                            xT_sb[ke][:, ng * NQ:(ng + 1) * NQ],
                            start=(ke == 0), stop=(ke == EK - 1))
                for t, (i, ng) in enumerate(tiles):
                    copy_alt(dsts[i][:, ng * NQ:(ng + 1) * NQ], pss[t])

            # ---- phase A: k,v projection ----
            proj_pass([(wkv_sb, 0), (wkv_sb, P)], [kT, vT])

            # ---- phase B1: q0,q1 projection ----
            proj_pass([(wq_sb, 0), (wq_sb, P)], [qT_sb[0], qT_sb[1]])

            # ---- V transposes to [S, D] (emitted in two groups) ----
            def transpose_steps(kcs):
                def mk(kc):
                    def step():
                        pst = psp.tile([P, P], F16, tag="ps", name="pst")
                        nc.tensor.transpose(
                            pst, vT[:, kc * P:(kc + 1) * P], id_sb)
                        nc.vector.tensor_copy(v_sb[kc], pst)
                    return step
                for kc in kcs:
                    yield mk(kc)

            # ---- attention / out-proj step generators ----
            def attn_steps(g4, h, depth=3):
                """Thunk stream for one (q-block, head): score steps software
                pipelined `depth` ahead of the PV steps so the PE never waits
                on the exp chain, then an epilogue thunk."""
                kmax = 4 * g4 + 4
                qT_h = qT_sb[h]
                state = {}
                prs = {}

                def mk_score(kc):
                    def step():
                        lo = max(0, P * (kc - 4 * g4))
                        if kc == 0:
                            state["ps_o"] = psp.tile(
                                [P, NQ], F32, tag="ps", name="pso")
                            state["acc"] = accp.tile(
                                [P, NQ], F16, tag="acc", name="acc")
                        acc = state["acc"]
                        ps_s = psp.tile([P, NQ], F32, tag="ps", name="pss")
                        nc.tensor.matmul(
                            ps_s[:, lo:], kT[:, kc * P:(kc + 1) * P],
                            qT_h[:, g4 * NQ + lo:(g4 + 1) * NQ],
                            start=True, stop=True)
                        pr = prb.tile([P, NQ], F16, tag="pr", name="pr")
                        prs[kc] = pr
                        nc.scalar.activation(
                            pr[:, lo:], ps_s[:, lo:],
                            mybir.ActivationFunctionType.Exp, scale=SCALE)
                        if kc - 4 * g4 >= 0:
                            nc.gpsimd.tensor_mul(
                                pr[:, lo:lo + P], pr[:, lo:lo + P], tri_sb)
                        if kc == 0:
                            nc.vector.tensor_copy(acc, pr)
                        else:
                            nc.vector.tensor_add(
                                acc[:, lo:], acc[:, lo:], pr[:, lo:])
                    return step

                def mk_pv(kc):
                    def step():
                        lo = max(0, P * (kc - 4 * g4))
                        nc.tensor.matmul(
                            state["ps_o"][:, lo:], v_sb[kc], prs.pop(kc)[:, lo:],
                            start=(kc == 0), stop=(kc == kmax - 1))
                    return step

                for kc in range(kmax):
                    yield mk_score(kc)
                    if kc >= depth:
                        yield mk_pv(kc - depth)
                for kc in range(max(0, kmax - depth), kmax):
                    yield mk_pv(kc)

                def epilogue():
                    ssum = sump.tile([P, NQ], F32, tag="sum", name="ssum")
                    nc.gpsimd.partition_all_reduce(
                        ssum, state["acc"], channels=P,
                        reduce_op=bass_isa.ReduceOp.add)
                    rec = recp.tile([P, NQ], F32, tag="rec", name="rec")
                    nc.vector.reciprocal(rec, ssum)
                    nc.vector.tensor_mul(
                        attn_sb[h][:, g4 * NQ:(g4 + 1) * NQ],
                        state["ps_o"], rec)
                yield epilogue

            def outproj_steps(ng, dve_only=False):
                def mk_me(me):
                    def step():
                        ps = psp.tile([P, NQ], F32, tag="ps", name="psop")
                        for j in range(4):
                            nc.tensor.matmul(
                                ps, wo_sb[j][:, me * P:(me + 1) * P],
                                attn_sb[j][:, ng * NQ:(ng + 1) * NQ],
                                start=(j == 0), stop=(j == 3))
                        ot = outp.tile([P, NQ], F16, tag="out", name="ot")
                        if dve_only:
                            nc.vector.tensor_copy(ot, ps)
                        else:
                            copy_alt(ot, ps)
                        nc.sync.dma_start(
                            out=outT[me * P:(me + 1) * P,
                                     ng * NQ:(ng + 1) * NQ],
                            in_=ot)
                    return step
                for me in range(E // P):
                    yield mk_me(me)

            def interleave(main, filler, every, offset=0):
                """Run all steps of `main`, inserting one `filler` step after
                every `every` main steps (starting after `offset` steps);
                flush remaining filler at the end."""
                import itertools
                i = 0
                for s in main:
                    s()
                    i += 1
                    if i > offset and (i - offset) % every == 0:
                        f = next(filler, None)
                        if f is not None:
                            f()
                for f in filler:
                    f()

            def chain(*gens):
                for g in gens:
                    yield from g

            # ---- phase B2 (q2,q3) with attention g4=0 interleaved ----
            # two 4-tile subpasses so attention tiles still get PSUM slots
            def b2_subpass_steps(h):
                pss = [psp.tile([P, NQ], F32, tag="ps", name="psb2")
                       for _ in range(4)]
                for ke in range(EK):
                    def step(ke=ke):
                        for ng in range(4):
                            nc.tensor.matmul(
                                pss[ng],
                                wq_sb[ke][:, h * P:(h + 1) * P],
                                xT_sb[ke][:, ng * NQ:(ng + 1) * NQ],
                                start=(ke == 0), stop=(ke == EK - 1))
                    yield step

                def copies():
                    for ng in range(4):
                        copy_alt(qT_sb[h][:, ng * NQ:(ng + 1) * NQ], pss[ng])
                yield copies

            # transposes for the first 4 key blocks precede attention g4=0
            for s in transpose_steps(range(4)):
                s()
            interleave(b2_subpass_steps(3),
                       chain(attn_steps(0, 0, depth=2),
                             attn_steps(0, 1, depth=2)), 1)
            interleave(b2_subpass_steps(2),
                       chain(attn_steps(0, 3, depth=2),
                             transpose_steps(range(4, 16))), 1)
            for s in attn_steps(0, 2, depth=2):
                s()

            # ---- attention g4 blocks interleaved with available out-proj
            # order: big blocks first so the tail is pure PE out-proj work
            def attn_g4(g4):
                for h in range(HPC):
                    yield from attn_steps(g4, h)

            interleave(attn_g4(3), outproj_steps(0, dve_only=True), 7,
                       offset=10)
            interleave(attn_g4(2), outproj_steps(3), 5, offset=6)
            interleave(attn_g4(1), outproj_steps(2), 4, offset=6)
            for s in outproj_steps(1):
                s()
    nc.compile()
    return nc


def _get_nc():
    if "nc" not in _CACHE:
        _CACHE["nc"] = _build()
    return _CACHE["nc"]


def kernel(x, Wq, Wk, Wv, Wo, _trace=False, _tmpdir=None):
    x = np.asarray(x, np.float32)
    Wq, Wk, Wv, Wo = (np.asarray(a, np.float32) for a in (Wq, Wk, Wv, Wo))
    nc = _get_nc()
    F16NP = np.float16
    ident = np.eye(P, dtype=F16NP)
    tri = np.triu(np.ones((P, P), np.float32)).astype(F16NP)
    from concurrent.futures import ThreadPoolExecutor
    with ThreadPoolExecutor(8) as tp:
        xT_f16 = list(tp.map(
            lambda b: np.ascontiguousarray(x[b].T).astype(F16NP),
            range(B)))

        def _core(c):
            b, g = c // 4, c % 4
            wkv = np.concatenate(
                [Wk[128 * g:128 * (g + 1)],
                 Wv[128 * g:128 * (g + 1)]], axis=0)
            return {
                "xT": xT_f16[b],
                "wqT": np.ascontiguousarray(
                    Wq[512 * g:512 * (g + 1)].T).astype(F16NP),
                "wkvT": np.ascontiguousarray(wkv.T).astype(F16NP),
                "woT": np.ascontiguousarray(
                    Wo[:, 512 * g:512 * (g + 1)].T).astype(F16NP),
                "ident": ident, "tri": tri,
            }
        in_maps = list(tp.map(_core, range(8)))
    res = bass_utils.run_bass_kernel_spmd(
        nc, in_maps, core_ids=list(range(8)), trace=_trace, tmpdir=_tmpdir)
    out = np.zeros((B, S, E), np.float32)
    for c in range(8):
        out[c // 4] += res.results[c]["outT"].T.astype(np.float32)
    if _trace:
        return out, res
    return out
